# revision 16
# baseline (speedup 1.0000x reference)
"""ArcFace loss on 8 TRN2 NeuronCores, sharded along the batch dim B.

Each core takes 64 rows x all 100k classes (25.6 MB) so the whole loss
for those rows is computed locally — no inter-core collective at all.

Per-core layout: the 64 rows are split into column halves so all 128
SBUF partitions stream: partition 2r+h holds row r, columns
[h*50000, (h+1)*50000). ScalarE computes exp(64c - 64) with per-row
accum_out row sums in a single HBM pass. Per-chunk accumulating
TensorE matmuls against a [128, 64] pair-fold matrix turn the
per-partition half-sums into per-row sums as the stream progresses.

The runtime splits a transfer's descriptors over the 16 DMA queues in
groups of ceil(n/16), restarting at queue 0 for every dma_start.
Measured per-queue service rates are uneven (queue 15 ~16% slower,
queues 0-3 ~4% slower), so a plain [0:128] stream leaves queue 15
straggling ~11us past the others. Rebalance by byte placement: FULL
chunks [0:128, w] spread over all 16 queues; the trailing columns go
through COMBINED chunks = two DMAs into one tile — [0:120] (n=120 ->
queues 0-14 only) plus [120:128] (n=8 -> queues 0-7) — processed by a
single full-width exp.

Since cosine <= 1, logits are <= 64, so exp(64c - 64) <= 1 and the
max-pass of logsumexp is unnecessary: logZ = 64 + log(sum). The margin
at the target column is folded in by correcting its exp contribution:
loss_r = 64 + ln(e^{-tS_r} * (sum_r + delta_r)) with
delta = e^{S*phi-64} - e^{S*c-64} and tS = S*phi, evaluated by one
Ln activation with per-partition scale/bias. A final ones-matmul sums
the 64 per-row losses; the host adds the 8 per-core partials.
"""

import math

import numpy as np

import concourse.bacc as bacc
import concourse.bass as bass
import concourse.mybir as mybir
import concourse.tile as tile
from concourse.bass_utils import run_bass_kernel_spmd

# ArcFace constants (match the reference)
S = 64.0
M = 0.5
COS_M = math.cos(M)
SIN_M = math.sin(M)
TH = math.cos(math.pi - M)
MM = math.sin(math.pi - M) * M
EPS = 1e-07

B, C = 512, 100000
NCORES = 8
RPC = B // NCORES  # 64 rows per core
HALF = C // 2  # 50000 columns per partition-row
P = 128
SHIFT = 64.0  # exp(S*c - SHIFT) keeps everything <= 1 since c in [0, 1)

FULL_PLAN = [5975] * 6 + [5950] + [1400, 600, 400]  # 44200 cols, all 16 queues
COMB_PLAN = [2000, 1500, 1200, 700, 400]  # 5800 cols, queues 0-14 + 0-7
FTOT = sum(FULL_PLAN)
assert FTOT + sum(COMB_PLAN) == HALF
NCH = len(FULL_PLAN) + len(COMB_PLAN)
FC_MAX = max(FULL_PLAN)

F32 = mybir.dt.float32
I32 = mybir.dt.int32


def _patch_act_tables():
    """Make natural_log_exp_and_others the only provider of Exp/Ln so the
    table-load pass emits a single ACT_TABLE_LOAD instead of thrashing
    between the exp-only and ln-only sets."""
    import concourse.hw_specs as hw_specs

    orig = hw_specs.get_activation_tables
    if getattr(orig, "_arcface_patched", False):
        return

    def patched(arch):
        tabs = {k: set(v) for k, v in orig(arch).items()}
        for name, fns in tabs.items():
            if name != "natural_log_exp_and_others":
                fns.discard(mybir.ActivationFunctionType.Exp)
                fns.discard(mybir.ActivationFunctionType.Ln)
        return tabs

    patched._arcface_patched = True
    hw_specs.get_activation_tables = patched
    bacc.get_activation_tables = patched


def build_nc():
    _patch_act_tables()
    nc = bacc.Bacc(None)
    cos_p = nc.declare_dram_parameter("cosine", [RPC, C], F32, isOutput=False)
    gidx_p = nc.declare_dram_parameter("gidx", [RPC, 1], I32, isOutput=False)
    out_p = nc.declare_dram_parameter("out", [1, 1], F32, isOutput=True)

    # [128, 50000]: partition 2r+h = row r, column half h
    cos_r = cos_p[:].rearrange("r (h c) -> (r h) c", h=2)
    cos_flat = cos_p[:].rearrange("r (c o) -> (r c) o", o=1)

    with tile.TileContext(nc) as tc:
        with (
            tc.tile_pool(name="data", bufs=4) as data_pool,
            tc.tile_pool(name="expp", bufs=2) as exp_pool,
            tc.tile_pool(name="small", bufs=1) as small,
            tc.tile_pool(name="psum", bufs=1, space="PSUM") as psum,
        ):
            # bias operand for exp(S*x - SHIFT) activations
            nbias = small.tile([P, 1], F32)
            nc.gpsimd.memset(nbias[:], -SHIFT)
            # dummy activation: pulls the ACT table load to the start of
            # the kernel instead of gating the first streaming exp
            warm_act = small.tile([P, 1], F32)
            nc.scalar.activation(
                out=warm_act[:], in_=nbias[:], func=mybir.ActivationFunctionType.Exp
            )

            # pair-fold matrix W[p, f] = (p >> 1) == f, so W.T @ v gives
            # v[2f] + v[2f+1] on partitions 0..63
            ji = small.tile([P, RPC], I32)
            nc.gpsimd.iota(ji[:], pattern=[[1, RPC]], base=0, channel_multiplier=0)
            pi = small.tile([P, 1], I32)
            nc.gpsimd.iota(pi[:], pattern=[[1, 1]], base=0, channel_multiplier=1)
            nc.vector.tensor_scalar(
                out=pi[:], in0=pi[:], scalar1=1, scalar2=None,
                op0=mybir.AluOpType.arith_shift_right,
            )
            pf = small.tile([P, 1], F32)
            nc.vector.tensor_copy(pf[:], pi[:])
            wfold = small.tile([P, RPC], F32)
            nc.vector.tensor_scalar(
                out=wfold[:], in0=ji[:], scalar1=pf[:], scalar2=None,
                op0=mybir.AluOpType.is_equal,
            )
            ones64 = small.tile([RPC, 1], F32)
            nc.gpsimd.memset(ones64[:], 1.0)

            # ---- target gather + margin math on [64, 1] tensors
            idx_sb = small.tile([RPC, 1], I32)
            nc.gpsimd.dma_start(out=idx_sb[:], in_=gidx_p[:])
            gc = small.tile([RPC, 1], F32)
            nc.gpsimd.indirect_dma_start(
                out=gc[:],
                out_offset=None,
                in_=cos_flat,
                in_offset=bass.IndirectOffsetOnAxis(ap=idx_sb[:], axis=0),
            )

            cg = small.tile([RPC, 1], F32)
            nc.vector.tensor_scalar(
                out=cg[:], in0=gc[:], scalar1=1.0 - EPS, scalar2=-1.0 + EPS,
                op0=mybir.AluOpType.min, op1=mybir.AluOpType.max,
            )
            # om = 1 - c^2  (via (c*c)*-1 + 1)
            om = small.tile([RPC, 1], F32)
            nc.vector.tensor_tensor(out=om[:], in0=cg[:], in1=cg[:], op=mybir.AluOpType.mult)
            nc.vector.tensor_scalar(
                out=om[:], in0=om[:], scalar1=-1.0, scalar2=1.0,
                op0=mybir.AluOpType.mult, op1=mybir.AluOpType.add,
            )
            # sine = exp(0.5 * ln(om)) — stays in the exp/ln table set
            sine = small.tile([RPC, 1], F32)
            nc.scalar.activation(out=sine[:], in_=om[:], func=mybir.ActivationFunctionType.Ln)
            nc.scalar.activation(
                out=sine[:], in_=sine[:], func=mybir.ActivationFunctionType.Exp, scale=0.5
            )
            # phi = c*COS_M - sine*SIN_M
            phi = small.tile([RPC, 1], F32)
            t1 = small.tile([RPC, 1], F32)
            nc.vector.tensor_scalar(
                out=t1[:], in0=sine[:], scalar1=SIN_M, scalar2=None, op0=mybir.AluOpType.mult
            )
            nc.vector.scalar_tensor_tensor(
                out=phi[:], in0=cg[:], scalar=COS_M, in1=t1[:],
                op0=mybir.AluOpType.mult, op1=mybir.AluOpType.subtract,
            )
            # phi = where(c > TH, phi, c - MM)
            gt = small.tile([RPC, 1], F32)
            nc.vector.tensor_scalar(
                out=gt[:], in0=cg[:], scalar1=TH, scalar2=None, op0=mybir.AluOpType.is_gt
            )
            cmm = small.tile([RPC, 1], F32)
            nc.vector.tensor_scalar(
                out=cmm[:], in0=cg[:], scalar1=MM, scalar2=None, op0=mybir.AluOpType.subtract
            )
            d = small.tile([RPC, 1], F32)
            nc.vector.tensor_tensor(out=d[:], in0=phi[:], in1=cmm[:], op=mybir.AluOpType.subtract)
            nc.vector.tensor_tensor(out=d[:], in0=d[:], in1=gt[:], op=mybir.AluOpType.mult)
            nc.vector.tensor_tensor(out=phi[:], in0=cmm[:], in1=d[:], op=mybir.AluOpType.add)

            # enT = e^(-S*phi); delta2 = (e^(S*phi-64) - e^(S*c-64)) * enT
            # so that ln(enT*rowsum + delta2) = ln(sum_r + delta) - S*phi
            enT = small.tile([RPC, 1], F32)
            nc.scalar.activation(
                out=enT[:], in_=phi[:], func=mybir.ActivationFunctionType.Exp, scale=-S
            )
            e_phi = small.tile([RPC, 1], F32)
            e_c = small.tile([RPC, 1], F32)
            nc.scalar.activation(
                out=e_phi[:], in_=phi[:], func=mybir.ActivationFunctionType.Exp,
                scale=S, bias=nbias[0:RPC, :],
            )
            nc.scalar.activation(
                out=e_c[:], in_=cg[:], func=mybir.ActivationFunctionType.Exp,
                scale=S, bias=nbias[0:RPC, :],
            )
            delta2 = small.tile([RPC, 1], F32)
            nc.vector.tensor_tensor(
                out=delta2[:], in0=e_phi[:], in1=e_c[:], op=mybir.AluOpType.subtract
            )
            nc.vector.tensor_tensor(
                out=delta2[:], in0=delta2[:], in1=enT[:], op=mybir.AluOpType.mult
            )

            # ---- main streaming pass: exp + per-partition row-sum accum,
            # folded per chunk into a PSUM [64, 1] by accumulating matmuls
            sums = small.tile([P, NCH], F32)
            folded = psum.tile([RPC, 1], F32)

            # Combined chunks go early so their exps overlap the stream;
            # the tapered full chunks come last to keep the final
            # (DMA-gated) exp short.
            plan = []
            off = 0
            for w in FULL_PLAN[:1]:
                plan.append((off, w, False))
                off += w
            boff = FTOT
            for w in COMB_PLAN:
                plan.append((boff, w, True))
                boff += w
            assert boff == HALF
            for w in FULL_PLAN[1:]:
                plan.append((off, w, False))
                off += w
            assert off == FTOT

            for i, (coff, w, comb) in enumerate(plan):
                dt = data_pool.tile([P, FC_MAX], F32, tag="data")
                if comb:
                    nc.sync.dma_start(
                        out=dt[0:120, 0:w], in_=cos_r[0:120, coff : coff + w]
                    )
                    nc.sync.dma_start(
                        out=dt[120:128, 0:w], in_=cos_r[120:128, coff : coff + w]
                    )
                else:
                    nc.sync.dma_start(out=dt[:, 0:w], in_=cos_r[:, coff : coff + w])
                ev = exp_pool.tile([P, FC_MAX], F32, tag="exp")
                nc.scalar.activation(
                    out=ev[:, 0:w], in_=dt[:, 0:w],
                    func=mybir.ActivationFunctionType.Exp,
                    scale=S, bias=nbias[:],
                    accum_out=sums[:, i : i + 1],
                )
                nc.tensor.matmul(
                    folded[:], wfold[:], sums[:, i : i + 1],
                    start=(i == 0), stop=(i == NCH - 1),
                )

            # ---- loss_r = ln(enT*rowsum + delta2); total = ones.T @ loss
            logv = small.tile([RPC, 1], F32)
            nc.scalar.activation(
                out=logv[:], in_=folded[:], func=mybir.ActivationFunctionType.Ln,
                scale=enT[:], bias=delta2[:],
            )
            tot = psum.tile([1, 1], F32)
            nc.tensor.matmul(tot[:], ones64[:], logv[:], start=True, stop=True)
            res = small.tile([1, 1], F32)
            nc.scalar.copy(res[:], tot[:])
            nc.sync.dma_start(out=out_p[:], in_=res[:])

    nc.finalize()
    return nc


_CACHE = {}


def _get_nc():
    if "nc" not in _CACHE:
        _CACHE["nc"] = build_nc()
    return _CACHE["nc"]


def make_in_maps(cosine: np.ndarray, labels: np.ndarray):
    labels = np.asarray(labels).astype(np.int64)
    rows = np.arange(RPC, dtype=np.int64)
    in_maps = []
    for m in range(NCORES):
        lo = m * RPC
        gidx = (rows * C + labels[lo : lo + RPC]).astype(np.int32).reshape(RPC, 1)
        in_maps.append({"cosine": cosine[lo : lo + RPC], "gidx": gidx})
    return in_maps


def kernel(cosine: np.ndarray, labels: np.ndarray, _trace: bool = False):
    nc = _get_nc()
    in_maps = make_in_maps(np.asarray(cosine, dtype=np.float32), labels)
    res = run_bass_kernel_spmd(
        nc, in_maps, core_ids=list(range(NCORES)), trace=_trace
    )
    parts = [np.asarray(r["out"], dtype=np.float64).reshape(()) for r in res.results]
    out = np.float32(SHIFT + sum(parts) / B)
    out = np.asarray(out, dtype=np.float32).reshape(())
    if _trace:
        return out, res
    return out


# revision 18
# speedup vs baseline: 1.0463x; 1.0463x over previous
"""ArcFace loss on 8 TRN2 NeuronCores, sharded along the batch dim B.

Each core takes 64 rows x all 100k classes (25.6 MB) so the whole loss
for those rows is computed locally — no inter-core collective at all.

Per-core layout: the 64 rows are split into column halves so all 128
SBUF partitions stream: partition 2r+h holds row r, columns
[h*50000, (h+1)*50000). ScalarE computes exp(64c - 64) with per-row
accum_out row sums in a single HBM pass. Per-chunk accumulating
TensorE matmuls against a [128, 64] pair-fold matrix turn the
per-partition half-sums into per-row sums as the stream progresses.

The runtime splits a transfer's descriptors over the 16 DMA queues in
groups of ceil(n/16), restarting at queue 0 for every dma_start.
Measured per-queue service rates are uneven (queue 15 ~16% slower,
queues 0-3 ~4% slower), so a plain [0:128] stream leaves queue 15
straggling ~11us past the others. Rebalance by byte placement: FULL
chunks [0:128, w] spread over all 16 queues; the trailing columns go
through COMBINED chunks = two DMAs into one tile — [0:120] (n=120 ->
queues 0-14 only) plus [120:128] (n=8 -> queues 0-7) — processed by a
single full-width exp.

Since cosine <= 1, logits are <= 64, so exp(64c - 64) <= 1 and the
max-pass of logsumexp is unnecessary: logZ = 64 + log(sum). The margin
at the target column is folded in by correcting its exp contribution:
loss_r = 64 + ln(e^{-tS_r} * (sum_r + delta_r)) with
delta = e^{S*phi-64} - e^{S*c-64} and tS = S*phi, evaluated by one
Ln activation with per-partition scale/bias. A final ones-matmul sums
the 64 per-row losses; the host adds the 8 per-core partials.
"""

import math

import numpy as np

import concourse.bacc as bacc
import concourse.bass as bass
import concourse.mybir as mybir
import concourse.tile as tile
from concourse.bass_utils import run_bass_kernel_spmd

# ArcFace constants (match the reference)
S = 64.0
M = 0.5
COS_M = math.cos(M)
SIN_M = math.sin(M)
TH = math.cos(math.pi - M)
MM = math.sin(math.pi - M) * M
EPS = 1e-07

B, C = 512, 100000
NCORES = 8
RPC = B // NCORES  # 64 rows per core
HALF = C // 2  # 50000 columns per partition-row
P = 128
SHIFT = 64.0  # exp(S*c - SHIFT) keeps everything <= 1 since c in [0, 1)

FULL_PLAN = [3800] * 11 + [1400, 600, 400]  # 44200 cols, all 16 queues
COMB_PLAN = [2000, 1500, 1200, 700, 400]  # 5800 cols, queues 0-14 + 0-7
FTOT = sum(FULL_PLAN)
assert FTOT + sum(COMB_PLAN) == HALF
NCH = len(FULL_PLAN) + len(COMB_PLAN)
FC_MAX = max(FULL_PLAN)

F32 = mybir.dt.float32
I32 = mybir.dt.int32


def _patch_act_tables():
    """Make natural_log_exp_and_others the only provider of Exp/Ln so the
    table-load pass emits a single ACT_TABLE_LOAD instead of thrashing
    between the exp-only and ln-only sets."""
    import concourse.hw_specs as hw_specs

    orig = hw_specs.get_activation_tables
    if getattr(orig, "_arcface_patched", False):
        return

    def patched(arch):
        tabs = {k: set(v) for k, v in orig(arch).items()}
        for name, fns in tabs.items():
            if name != "natural_log_exp_and_others":
                fns.discard(mybir.ActivationFunctionType.Exp)
                fns.discard(mybir.ActivationFunctionType.Ln)
        return tabs

    patched._arcface_patched = True
    hw_specs.get_activation_tables = patched
    bacc.get_activation_tables = patched


def build_nc():
    _patch_act_tables()
    nc = bacc.Bacc(None)
    cos_p = nc.declare_dram_parameter("cosine", [RPC, C], F32, isOutput=False)
    gidx_p = nc.declare_dram_parameter("gidx", [RPC, 1], I32, isOutput=False)
    out_p = nc.declare_dram_parameter("out", [1, 1], F32, isOutput=True)

    # [128, 50000]: partition 2r+h = row r, column half h
    cos_r = cos_p[:].rearrange("r (h c) -> (r h) c", h=2)
    cos_flat = cos_p[:].rearrange("r (c o) -> (r c) o", o=1)

    with tile.TileContext(nc) as tc:
        with (
            tc.tile_pool(name="data", bufs=6) as data_pool,
            tc.tile_pool(name="expp", bufs=2) as exp_pool,
            tc.tile_pool(name="small", bufs=1) as small,
            tc.tile_pool(name="psum", bufs=1, space="PSUM") as psum,
        ):
            # bias operand for exp(S*x - SHIFT) activations
            nbias = small.tile([P, 1], F32)
            nc.gpsimd.memset(nbias[:], -SHIFT)
            # dummy activation: pulls the ACT table load to the start of
            # the kernel instead of gating the first streaming exp
            warm_act = small.tile([P, 1], F32)
            nc.scalar.activation(
                out=warm_act[:], in_=nbias[:], func=mybir.ActivationFunctionType.Exp
            )

            # pair-fold matrix W[p, f] = (p >> 1) == f, so W.T @ v gives
            # v[2f] + v[2f+1] on partitions 0..63
            ji = small.tile([P, RPC], I32)
            nc.gpsimd.iota(ji[:], pattern=[[1, RPC]], base=0, channel_multiplier=0)
            pi = small.tile([P, 1], I32)
            nc.gpsimd.iota(pi[:], pattern=[[1, 1]], base=0, channel_multiplier=1)
            nc.vector.tensor_scalar(
                out=pi[:], in0=pi[:], scalar1=1, scalar2=None,
                op0=mybir.AluOpType.arith_shift_right,
            )
            pf = small.tile([P, 1], F32)
            nc.vector.tensor_copy(pf[:], pi[:])
            wfold = small.tile([P, RPC], F32)
            nc.vector.tensor_scalar(
                out=wfold[:], in0=ji[:], scalar1=pf[:], scalar2=None,
                op0=mybir.AluOpType.is_equal,
            )
            ones64 = small.tile([RPC, 1], F32)
            nc.gpsimd.memset(ones64[:], 1.0)

            # ---- target gather + margin math on [64, 1] tensors
            idx_sb = small.tile([RPC, 1], I32)
            nc.gpsimd.dma_start(out=idx_sb[:], in_=gidx_p[:])
            gc = small.tile([RPC, 1], F32)
            nc.gpsimd.indirect_dma_start(
                out=gc[:],
                out_offset=None,
                in_=cos_flat,
                in_offset=bass.IndirectOffsetOnAxis(ap=idx_sb[:], axis=0),
            )

            cg = small.tile([RPC, 1], F32)
            nc.vector.tensor_scalar(
                out=cg[:], in0=gc[:], scalar1=1.0 - EPS, scalar2=-1.0 + EPS,
                op0=mybir.AluOpType.min, op1=mybir.AluOpType.max,
            )
            # om = 1 - c^2  (via (c*c)*-1 + 1)
            om = small.tile([RPC, 1], F32)
            nc.vector.tensor_tensor(out=om[:], in0=cg[:], in1=cg[:], op=mybir.AluOpType.mult)
            nc.vector.tensor_scalar(
                out=om[:], in0=om[:], scalar1=-1.0, scalar2=1.0,
                op0=mybir.AluOpType.mult, op1=mybir.AluOpType.add,
            )
            # sine = exp(0.5 * ln(om)) — stays in the exp/ln table set
            sine = small.tile([RPC, 1], F32)
            nc.scalar.activation(out=sine[:], in_=om[:], func=mybir.ActivationFunctionType.Ln)
            nc.scalar.activation(
                out=sine[:], in_=sine[:], func=mybir.ActivationFunctionType.Exp, scale=0.5
            )
            # phi = c*COS_M - sine*SIN_M
            phi = small.tile([RPC, 1], F32)
            t1 = small.tile([RPC, 1], F32)
            nc.vector.tensor_scalar(
                out=t1[:], in0=sine[:], scalar1=SIN_M, scalar2=None, op0=mybir.AluOpType.mult
            )
            nc.vector.scalar_tensor_tensor(
                out=phi[:], in0=cg[:], scalar=COS_M, in1=t1[:],
                op0=mybir.AluOpType.mult, op1=mybir.AluOpType.subtract,
            )
            # phi = where(c > TH, phi, c - MM)
            gt = small.tile([RPC, 1], F32)
            nc.vector.tensor_scalar(
                out=gt[:], in0=cg[:], scalar1=TH, scalar2=None, op0=mybir.AluOpType.is_gt
            )
            cmm = small.tile([RPC, 1], F32)
            nc.vector.tensor_scalar(
                out=cmm[:], in0=cg[:], scalar1=MM, scalar2=None, op0=mybir.AluOpType.subtract
            )
            d = small.tile([RPC, 1], F32)
            nc.vector.tensor_tensor(out=d[:], in0=phi[:], in1=cmm[:], op=mybir.AluOpType.subtract)
            nc.vector.tensor_tensor(out=d[:], in0=d[:], in1=gt[:], op=mybir.AluOpType.mult)
            nc.vector.tensor_tensor(out=phi[:], in0=cmm[:], in1=d[:], op=mybir.AluOpType.add)

            # enT = e^(-S*phi); delta2 = (e^(S*phi-64) - e^(S*c-64)) * enT
            # so that ln(enT*rowsum + delta2) = ln(sum_r + delta) - S*phi
            enT = small.tile([RPC, 1], F32)
            nc.scalar.activation(
                out=enT[:], in_=phi[:], func=mybir.ActivationFunctionType.Exp, scale=-S
            )
            e_phi = small.tile([RPC, 1], F32)
            e_c = small.tile([RPC, 1], F32)
            nc.scalar.activation(
                out=e_phi[:], in_=phi[:], func=mybir.ActivationFunctionType.Exp,
                scale=S, bias=nbias[0:RPC, :],
            )
            nc.scalar.activation(
                out=e_c[:], in_=cg[:], func=mybir.ActivationFunctionType.Exp,
                scale=S, bias=nbias[0:RPC, :],
            )
            delta2 = small.tile([RPC, 1], F32)
            nc.vector.tensor_tensor(
                out=delta2[:], in0=e_phi[:], in1=e_c[:], op=mybir.AluOpType.subtract
            )
            nc.vector.tensor_tensor(
                out=delta2[:], in0=delta2[:], in1=enT[:], op=mybir.AluOpType.mult
            )

            # ---- main streaming pass: exp + per-partition row-sum accum,
            # folded per chunk into a PSUM [64, 1] by accumulating matmuls
            sums = small.tile([P, NCH], F32)
            folded = psum.tile([RPC, 1], F32)

            # Combined chunks go early so their exps overlap the stream;
            # the tapered full chunks come last to keep the final
            # (DMA-gated) exp short.
            plan = []
            off = 0
            for w in FULL_PLAN[:1]:
                plan.append((off, w, False))
                off += w
            boff = FTOT
            for w in COMB_PLAN:
                plan.append((boff, w, True))
                boff += w
            assert boff == HALF
            for w in FULL_PLAN[1:]:
                plan.append((off, w, False))
                off += w
            assert off == FTOT

            for i, (coff, w, comb) in enumerate(plan):
                dt = data_pool.tile([P, FC_MAX], F32, tag="data")
                if comb:
                    nc.sync.dma_start(
                        out=dt[0:120, 0:w], in_=cos_r[0:120, coff : coff + w]
                    )
                    nc.sync.dma_start(
                        out=dt[120:128, 0:w], in_=cos_r[120:128, coff : coff + w]
                    )
                else:
                    nc.sync.dma_start(out=dt[:, 0:w], in_=cos_r[:, coff : coff + w])
                ev = exp_pool.tile([P, FC_MAX], F32, tag="exp")
                nc.scalar.activation(
                    out=ev[:, 0:w], in_=dt[:, 0:w],
                    func=mybir.ActivationFunctionType.Exp,
                    scale=S, bias=nbias[:],
                    accum_out=sums[:, i : i + 1],
                )
                nc.tensor.matmul(
                    folded[:], wfold[:], sums[:, i : i + 1],
                    start=(i == 0), stop=(i == NCH - 1),
                )

            # ---- loss_r = ln(enT*rowsum + delta2); total = ones.T @ loss
            logv = small.tile([RPC, 1], F32)
            nc.scalar.activation(
                out=logv[:], in_=folded[:], func=mybir.ActivationFunctionType.Ln,
                scale=enT[:], bias=delta2[:],
            )
            tot = psum.tile([1, 1], F32)
            nc.tensor.matmul(tot[:], ones64[:], logv[:], start=True, stop=True)
            res = small.tile([1, 1], F32)
            nc.scalar.copy(res[:], tot[:])
            nc.sync.dma_start(out=out_p[:], in_=res[:])

    nc.finalize()
    return nc


_CACHE = {}


def _get_nc():
    if "nc" not in _CACHE:
        _CACHE["nc"] = build_nc()
    return _CACHE["nc"]


def make_in_maps(cosine: np.ndarray, labels: np.ndarray):
    labels = np.asarray(labels).astype(np.int64)
    rows = np.arange(RPC, dtype=np.int64)
    in_maps = []
    for m in range(NCORES):
        lo = m * RPC
        gidx = (rows * C + labels[lo : lo + RPC]).astype(np.int32).reshape(RPC, 1)
        in_maps.append({"cosine": cosine[lo : lo + RPC], "gidx": gidx})
    return in_maps


def kernel(cosine: np.ndarray, labels: np.ndarray, _trace: bool = False):
    nc = _get_nc()
    in_maps = make_in_maps(np.asarray(cosine, dtype=np.float32), labels)
    res = run_bass_kernel_spmd(
        nc, in_maps, core_ids=list(range(NCORES)), trace=_trace
    )
    parts = [np.asarray(r["out"], dtype=np.float64).reshape(()) for r in res.results]
    out = np.float32(SHIFT + sum(parts) / B)
    out = np.asarray(out, dtype=np.float32).reshape(())
    if _trace:
        return out, res
    return out


# revision 20
# speedup vs baseline: 1.1168x; 1.0674x over previous
"""ArcFace loss on 8 TRN2 NeuronCores, sharded along the batch dim B.

Each core takes 64 rows x all 100k classes (25.6 MB) so the whole loss
for those rows is computed locally — no inter-core collective at all.

Per-core layout: the 64 rows are split into column halves so all 128
SBUF partitions stream: partition 2r+h holds row r, columns
[h*50000, (h+1)*50000). ScalarE computes exp(64c - 64) with per-row
accum_out row sums in a single HBM pass. Per-chunk accumulating
TensorE matmuls against a [128, 64] pair-fold matrix turn the
per-partition half-sums into per-row sums as the stream progresses.

The runtime splits a transfer's descriptors over the 16 DMA queues in
groups of ceil(n/16), restarting at queue 0 for every dma_start.
Measured per-queue service rates are uneven (queue 15 ~16% slower,
queues 0-3 ~4% slower), so a plain [0:128] stream leaves queue 15
straggling ~11us past the others. Rebalance by byte placement: FULL
chunks [0:128, w] spread over all 16 queues; the trailing columns go
through COMBINED chunks = two DMAs into one tile — [0:120] (n=120 ->
queues 0-14 only) plus [120:128] (n=8 -> queues 0-7) — processed by a
single full-width exp.

Since cosine <= 1, logits are <= 64, so exp(64c - 64) <= 1 and the
max-pass of logsumexp is unnecessary: logZ = 64 + log(sum). The margin
at the target column is folded in by correcting its exp contribution:
loss_r = 64 + ln(e^{-tS_r} * (sum_r + delta_r)) with
delta = e^{S*phi-64} - e^{S*c-64} and tS = S*phi, evaluated by one
Ln activation with per-partition scale/bias. A final ones-matmul sums
the 64 per-row losses; the host adds the 8 per-core partials.
"""

import math

import numpy as np

import concourse.bacc as bacc
import concourse.bass as bass
import concourse.mybir as mybir
import concourse.tile as tile
from concourse.bass_utils import run_bass_kernel_spmd

# ArcFace constants (match the reference)
S = 64.0
M = 0.5
COS_M = math.cos(M)
SIN_M = math.sin(M)
TH = math.cos(math.pi - M)
MM = math.sin(math.pi - M) * M
EPS = 1e-07

B, C = 512, 100000
NCORES = 8
RPC = B // NCORES  # 64 rows per core
HALF = C // 2  # 50000 columns per partition-row
P = 128
SHIFT = 64.0  # exp(S*c - SHIFT) keeps everything <= 1 since c in [0, 1)

FULL_PLAN = [3800] * 11 + [1800]  # 43600 cols, all 16 queues
# Trailing chunks avoid queue 15 so the slow queue retires early and the
# end-game chunk completions stay spaced; tapered so the final
# (DMA-gated) exps are short.
COMB_PLAN = [1800, 1500, 1200, 900, 600, 400]  # 6400 cols, queues 0-14 + 0-7
FTOT = sum(FULL_PLAN)
assert FTOT + sum(COMB_PLAN) == HALF
NCH = len(FULL_PLAN) + len(COMB_PLAN)
FC_MAX = max(FULL_PLAN)

F32 = mybir.dt.float32
I32 = mybir.dt.int32


def _patch_act_tables():
    """Make natural_log_exp_and_others the only provider of Exp/Ln so the
    table-load pass emits a single ACT_TABLE_LOAD instead of thrashing
    between the exp-only and ln-only sets."""
    import concourse.hw_specs as hw_specs

    orig = hw_specs.get_activation_tables
    if getattr(orig, "_arcface_patched", False):
        return

    def patched(arch):
        tabs = {k: set(v) for k, v in orig(arch).items()}
        for name, fns in tabs.items():
            if name != "natural_log_exp_and_others":
                fns.discard(mybir.ActivationFunctionType.Exp)
                fns.discard(mybir.ActivationFunctionType.Ln)
        return tabs

    patched._arcface_patched = True
    hw_specs.get_activation_tables = patched
    bacc.get_activation_tables = patched


def build_nc():
    _patch_act_tables()
    nc = bacc.Bacc(None)
    cos_p = nc.declare_dram_parameter("cosine", [RPC, C], F32, isOutput=False)
    gidx_p = nc.declare_dram_parameter("gidx", [RPC, 1], I32, isOutput=False)
    out_p = nc.declare_dram_parameter("out", [1, 1], F32, isOutput=True)

    # [128, 50000]: partition 2r+h = row r, column half h
    cos_r = cos_p[:].rearrange("r (h c) -> (r h) c", h=2)
    cos_flat = cos_p[:].rearrange("r (c o) -> (r c) o", o=1)

    with tile.TileContext(nc) as tc:
        with (
            tc.tile_pool(name="data", bufs=6) as data_pool,
            tc.tile_pool(name="expp", bufs=2) as exp_pool,
            tc.tile_pool(name="small", bufs=1) as small,
            tc.tile_pool(name="psum", bufs=1, space="PSUM") as psum,
        ):
            # bias operand for exp(S*x - SHIFT) activations
            nbias = small.tile([P, 1], F32)
            nc.gpsimd.memset(nbias[:], -SHIFT)
            # dummy activation: pulls the ACT table load to the start of
            # the kernel instead of gating the first streaming exp
            warm_act = small.tile([P, 1], F32)
            nc.scalar.activation(
                out=warm_act[:], in_=nbias[:], func=mybir.ActivationFunctionType.Exp
            )

            # pair-fold matrix W[p, f] = (p >> 1) == f, so W.T @ v gives
            # v[2f] + v[2f+1] on partitions 0..63
            ji = small.tile([P, RPC], I32)
            nc.gpsimd.iota(ji[:], pattern=[[1, RPC]], base=0, channel_multiplier=0)
            pi = small.tile([P, 1], I32)
            nc.gpsimd.iota(pi[:], pattern=[[1, 1]], base=0, channel_multiplier=1)
            nc.vector.tensor_scalar(
                out=pi[:], in0=pi[:], scalar1=1, scalar2=None,
                op0=mybir.AluOpType.arith_shift_right,
            )
            pf = small.tile([P, 1], F32)
            nc.vector.tensor_copy(pf[:], pi[:])
            wfold = small.tile([P, RPC], F32)
            nc.vector.tensor_scalar(
                out=wfold[:], in0=ji[:], scalar1=pf[:], scalar2=None,
                op0=mybir.AluOpType.is_equal,
            )
            ones64 = small.tile([RPC, 1], F32)
            nc.gpsimd.memset(ones64[:], 1.0)

            # ---- target gather + margin math on [64, 1] tensors
            idx_sb = small.tile([RPC, 1], I32)
            nc.gpsimd.dma_start(out=idx_sb[:], in_=gidx_p[:])
            gc = small.tile([RPC, 1], F32)
            nc.gpsimd.indirect_dma_start(
                out=gc[:],
                out_offset=None,
                in_=cos_flat,
                in_offset=bass.IndirectOffsetOnAxis(ap=idx_sb[:], axis=0),
            )

            cg = small.tile([RPC, 1], F32)
            nc.vector.tensor_scalar(
                out=cg[:], in0=gc[:], scalar1=1.0 - EPS, scalar2=-1.0 + EPS,
                op0=mybir.AluOpType.min, op1=mybir.AluOpType.max,
            )
            # om = 1 - c^2  (via (c*c)*-1 + 1)
            om = small.tile([RPC, 1], F32)
            nc.vector.tensor_tensor(out=om[:], in0=cg[:], in1=cg[:], op=mybir.AluOpType.mult)
            nc.vector.tensor_scalar(
                out=om[:], in0=om[:], scalar1=-1.0, scalar2=1.0,
                op0=mybir.AluOpType.mult, op1=mybir.AluOpType.add,
            )
            # sine = exp(0.5 * ln(om)) — stays in the exp/ln table set
            sine = small.tile([RPC, 1], F32)
            nc.scalar.activation(out=sine[:], in_=om[:], func=mybir.ActivationFunctionType.Ln)
            nc.scalar.activation(
                out=sine[:], in_=sine[:], func=mybir.ActivationFunctionType.Exp, scale=0.5
            )
            # phi = c*COS_M - sine*SIN_M
            phi = small.tile([RPC, 1], F32)
            t1 = small.tile([RPC, 1], F32)
            nc.vector.tensor_scalar(
                out=t1[:], in0=sine[:], scalar1=SIN_M, scalar2=None, op0=mybir.AluOpType.mult
            )
            nc.vector.scalar_tensor_tensor(
                out=phi[:], in0=cg[:], scalar=COS_M, in1=t1[:],
                op0=mybir.AluOpType.mult, op1=mybir.AluOpType.subtract,
            )
            # phi = where(c > TH, phi, c - MM)
            gt = small.tile([RPC, 1], F32)
            nc.vector.tensor_scalar(
                out=gt[:], in0=cg[:], scalar1=TH, scalar2=None, op0=mybir.AluOpType.is_gt
            )
            cmm = small.tile([RPC, 1], F32)
            nc.vector.tensor_scalar(
                out=cmm[:], in0=cg[:], scalar1=MM, scalar2=None, op0=mybir.AluOpType.subtract
            )
            d = small.tile([RPC, 1], F32)
            nc.vector.tensor_tensor(out=d[:], in0=phi[:], in1=cmm[:], op=mybir.AluOpType.subtract)
            nc.vector.tensor_tensor(out=d[:], in0=d[:], in1=gt[:], op=mybir.AluOpType.mult)
            nc.vector.tensor_tensor(out=phi[:], in0=cmm[:], in1=d[:], op=mybir.AluOpType.add)

            # enT = e^(-S*phi); delta2 = (e^(S*phi-64) - e^(S*c-64)) * enT
            # so that ln(enT*rowsum + delta2) = ln(sum_r + delta) - S*phi
            enT = small.tile([RPC, 1], F32)
            nc.scalar.activation(
                out=enT[:], in_=phi[:], func=mybir.ActivationFunctionType.Exp, scale=-S
            )
            e_phi = small.tile([RPC, 1], F32)
            e_c = small.tile([RPC, 1], F32)
            nc.scalar.activation(
                out=e_phi[:], in_=phi[:], func=mybir.ActivationFunctionType.Exp,
                scale=S, bias=nbias[0:RPC, :],
            )
            nc.scalar.activation(
                out=e_c[:], in_=cg[:], func=mybir.ActivationFunctionType.Exp,
                scale=S, bias=nbias[0:RPC, :],
            )
            delta2 = small.tile([RPC, 1], F32)
            nc.vector.tensor_tensor(
                out=delta2[:], in0=e_phi[:], in1=e_c[:], op=mybir.AluOpType.subtract
            )
            nc.vector.tensor_tensor(
                out=delta2[:], in0=delta2[:], in1=enT[:], op=mybir.AluOpType.mult
            )

            # ---- main streaming pass: exp + per-partition row-sum accum,
            # folded per chunk into a PSUM [64, 1] by accumulating matmuls
            sums = small.tile([P, NCH], F32)
            folded = psum.tile([RPC, 1], F32)

            plan = []
            off = 0
            for w in FULL_PLAN:
                plan.append((off, w, False))
                off += w
            for w in COMB_PLAN:
                plan.append((off, w, True))
                off += w
            assert off == HALF

            for i, (coff, w, comb) in enumerate(plan):
                dt = data_pool.tile([P, FC_MAX], F32, tag="data")
                if comb:
                    nc.sync.dma_start(
                        out=dt[0:120, 0:w], in_=cos_r[0:120, coff : coff + w]
                    )
                    nc.sync.dma_start(
                        out=dt[120:128, 0:w], in_=cos_r[120:128, coff : coff + w]
                    )
                else:
                    nc.sync.dma_start(out=dt[:, 0:w], in_=cos_r[:, coff : coff + w])
                ev = exp_pool.tile([P, FC_MAX], F32, tag="exp")
                nc.scalar.activation(
                    out=ev[:, 0:w], in_=dt[:, 0:w],
                    func=mybir.ActivationFunctionType.Exp,
                    scale=S, bias=nbias[:],
                    accum_out=sums[:, i : i + 1],
                )
                nc.tensor.matmul(
                    folded[:], wfold[:], sums[:, i : i + 1],
                    start=(i == 0), stop=(i == NCH - 1),
                )

            # ---- loss_r = ln(enT*rowsum + delta2); total = ones.T @ loss
            logv = small.tile([RPC, 1], F32)
            nc.scalar.activation(
                out=logv[:], in_=folded[:], func=mybir.ActivationFunctionType.Ln,
                scale=enT[:], bias=delta2[:],
            )
            tot = psum.tile([1, 1], F32)
            nc.tensor.matmul(tot[:], ones64[:], logv[:], start=True, stop=True)
            res = small.tile([1, 1], F32)
            nc.scalar.copy(res[:], tot[:])
            nc.sync.dma_start(out=out_p[:], in_=res[:])

    nc.finalize()
    return nc


_CACHE = {}


def _get_nc():
    if "nc" not in _CACHE:
        _CACHE["nc"] = build_nc()
    return _CACHE["nc"]


def make_in_maps(cosine: np.ndarray, labels: np.ndarray):
    labels = np.asarray(labels).astype(np.int64)
    rows = np.arange(RPC, dtype=np.int64)
    in_maps = []
    for m in range(NCORES):
        lo = m * RPC
        gidx = (rows * C + labels[lo : lo + RPC]).astype(np.int32).reshape(RPC, 1)
        in_maps.append({"cosine": cosine[lo : lo + RPC], "gidx": gidx})
    return in_maps


def kernel(cosine: np.ndarray, labels: np.ndarray, _trace: bool = False):
    nc = _get_nc()
    in_maps = make_in_maps(np.asarray(cosine, dtype=np.float32), labels)
    res = run_bass_kernel_spmd(
        nc, in_maps, core_ids=list(range(NCORES)), trace=_trace
    )
    parts = [np.asarray(r["out"], dtype=np.float64).reshape(()) for r in res.results]
    out = np.float32(SHIFT + sum(parts) / B)
    out = np.asarray(out, dtype=np.float32).reshape(())
    if _trace:
        return out, res
    return out
